# revision 7
# baseline (speedup 1.0000x reference)
"""Trainium2 Bass kernel for nn_MultiHeadAttention (B=2, S=2048, DM=1024, H=16).

Sharding: 8 cores = 2 (batch) x 4 (head-groups of 4 heads).
Each core computes QKV projections for its 4 heads' columns, attention in
transposed orientation (scores^T [key, query] so the softmax denominator is
produced by the same matmul chain that computes ctx), and a row-sharded
output projection. Host gathers: sums the 4 output-projection partials per
batch and un-transposes the attention weights (as a view).
"""

import os
import numpy as np

S = 2048
DM = 1024
H = 16
DH = 64
HPC = 4              # heads per core
COLS = HPC * DH      # 256 projection columns per core
P = 128
NKO = DM // P        # 8 contraction chunks for projections
NJT = S // P         # 16 key tiles
QC = 512             # query chunk (free dim per matmul)
NQC = S // QC        # 4
SCALE = 1.0 / np.sqrt(DH)

_CACHE = {}


def _build_nc():
    import concourse.tile as tile
    from concourse import bacc, mybir

    FP = mybir.dt.float32
    use_f32r = os.environ.get("MHA_MM_DT", "float32r") == "float32r"
    MM = mybir.dt.float32r if use_f32r else FP
    Exp = mybir.ActivationFunctionType.Exp
    Ident = mybir.ActivationFunctionType.Identity

    nc = bacc.Bacc("TRN2", target_bir_lowering=False, debug=False)

    QT = nc.dram_tensor("QT", [DM, S], FP, kind="ExternalInput").ap()
    KT = nc.dram_tensor("KT", [DM, S], FP, kind="ExternalInput").ap()
    VT = nc.dram_tensor("VT", [DM, S], FP, kind="ExternalInput").ap()
    Wq = nc.dram_tensor("Wq", [DM, COLS], FP, kind="ExternalInput").ap()
    Wk = nc.dram_tensor("Wk", [DM, COLS], FP, kind="ExternalInput").ap()
    Wv = nc.dram_tensor("Wv", [DM, COLS], FP, kind="ExternalInput").ap()
    Wo = nc.dram_tensor("Wo", [COLS, DM], FP, kind="ExternalInput").ap()
    bqi = nc.dram_tensor("bq", [COLS], FP, kind="ExternalInput").ap()
    bki = nc.dram_tensor("bk", [COLS], FP, kind="ExternalInput").ap()
    bvi = nc.dram_tensor("bv", [COLS], FP, kind="ExternalInput").ap()
    mask = nc.dram_tensor("mask", [S], FP, kind="ExternalInput").ap()
    ones = nc.dram_tensor("ones", [P, NJT * HPC], FP, kind="ExternalInput").ap()
    WTo = nc.dram_tensor("WT", [HPC, S, S], FP, kind="ExternalOutput").ap()
    OUT = nc.dram_tensor("OUT", [S, DM], FP, kind="ExternalOutput").ap()

    def mc_(ap):
        return ap.bitcast(MM) if use_f32r else ap

    fr_ = mc_  # producer-side cast: f32r matmul operands must be written as f32r

    with tile.TileContext(nc) as tc:
        with tc.tile_pool(name="persist", bufs=1) as persist:
            qT = persist.tile([P, 2, S], FP, tag="qT")
            kT = persist.tile([P, 2, S], FP, tag="kT")
            vA = persist.tile([P, NJT, HPC, DH + 1], FP, tag="vA")
            ctxT = persist.tile([P, 2, S], FP, tag="ctxT")
            wq_sb = persist.tile([P, NKO, COLS], FP, tag="wq")
            wk_sb = persist.tile([P, NKO, COLS], FP, tag="wk")
            wv_sb = persist.tile([P, NKO, COLS], FP, tag="wv")
            wo_sb = persist.tile([P, 2, DM], FP, tag="wo")
            bq_sb = persist.tile([P, 2], FP, tag="bq")
            bk_sb = persist.tile([P, 2], FP, tag="bk")
            bv_row = persist.tile([1, COLS], FP, tag="bvr")
            bv_bc = persist.tile([P, COLS], FP, tag="bvb")
            mbias = persist.tile([P, NJT], FP, tag="mb")

            nc.sync.dma_start(fr_(wq_sb[:]), fr_(Wq.rearrange("(ko p) c -> p ko c", p=P)))
            nc.sync.dma_start(fr_(wk_sb[:]), fr_(Wk.rearrange("(ko p) c -> p ko c", p=P)))
            nc.sync.dma_start(fr_(wv_sb[:]), fr_(Wv.rearrange("(ko p) c -> p ko c", p=P)))
            nc.sync.dma_start(fr_(wo_sb[:]), fr_(Wo.rearrange("(kc p) n -> p kc n", p=P)))
            nc.sync.dma_start(bq_sb[:], bqi.rearrange("(c p) -> p c", p=P))
            nc.sync.dma_start(bk_sb[:], bki.rearrange("(c p) -> p c", p=P))
            nc.sync.dma_start(bv_row[:], bvi[None, :])
            nc.sync.dma_start(mbias[:], mask.rearrange("(t p) -> p t", p=P))
            # fold the attention scale into q's bias and values
            nc.vector.tensor_scalar_mul(bq_sb[:], bq_sb[:], SCALE)
            nc.vector.tensor_scalar_mul(mbias[:], mbias[:], -1e9)
            nc.gpsimd.partition_broadcast(bv_bc[:], bv_row[0:1, :])
            nc.sync.dma_start(
                fr_(vA[:, :, :, DH]),
                fr_(ones.rearrange("p (t h) -> p t h", t=NJT)))

            # ---- Stage 1: projections ----
            with tc.tile_pool(name="pj_in", bufs=3) as pj_in, \
                 tc.tile_pool(name="psum1", bufs=8, space="PSUM") as psum1:
                for XT, w_sb, b_sb, scl, dst in (
                    (QT, wq_sb, bq_sb, SCALE, qT),
                    (KT, wk_sb, bk_sb, 1.0, kT),
                ):
                    psums = [[psum1.tile([P, QC], FP, tag="pj_psum",
                                         name=f"pj_psum_{i}_{j}")
                              for j in range(NQC)] for i in range(2)]
                    for ko in range(NKO):
                        xt = pj_in.tile([P, S], FP, tag="xt")
                        nc.sync.dma_start(fr_(xt[:]), fr_(XT[ko * P:(ko + 1) * P, :]))
                        for mcI in range(2):
                            lhs = mc_(w_sb[:, ko, mcI * P:(mcI + 1) * P])
                            for qc in range(NQC):
                                nc.tensor.matmul(
                                    psums[mcI][qc][:], lhs,
                                    mc_(xt[:, qc * QC:(qc + 1) * QC]),
                                    start=(ko == 0), stop=(ko == NKO - 1))
                    for mcI in range(2):
                        for qc in range(NQC):
                            nc.scalar.activation(
                                fr_(dst[:, mcI, qc * QC:(qc + 1) * QC]),
                                psums[mcI][qc][:], Ident,
                                bias=b_sb[:, mcI:mcI + 1], scale=scl)

                for sc in range(NJT):
                    vt = pj_in.tile([P, NKO, P], FP, tag="vt")
                    nc.sync.dma_start(
                        fr_(vt[:]),
                        fr_(VT[:, sc * P:(sc + 1) * P].rearrange(
                            "(ko p) s -> p ko s", p=P)))
                    ps = psum1.tile([P, QC], FP, tag="pj_psum")
                    for ko in range(NKO):
                        nc.tensor.matmul(
                            ps[:, :COLS], mc_(vt[:, ko, :]), mc_(wv_sb[:, ko, :]),
                            start=(ko == 0), stop=(ko == NKO - 1))
                    nc.vector.tensor_add(
                        fr_(vA[:, sc, :, 0:DH]),
                        ps[:, :COLS].rearrange("p (h d) -> p h d", d=DH),
                        bv_bc.rearrange("p (h d) -> p h d", d=DH))

            # ---- Stage 2: attention (transposed orientation) ----
            with tc.tile_pool(name="att_e", bufs=2) as e_pool, \
                 tc.tile_pool(name="att_sm", bufs=4) as sm_pool, \
                 tc.tile_pool(name="spsum", bufs=4, space="PSUM") as spool, \
                 tc.tile_pool(name="cpsum", bufs=2, space="PSUM") as cpool:
                for h in range(HPC):
                    pb = (h % 2) * DH
                    mcI = h // 2
                    for qc in range(NQC):
                        qs = slice(qc * QC, (qc + 1) * QC)
                        e = e_pool.tile([P, NJT, QC], FP, tag="e")
                        cps = cpool.tile([P, QC], FP, tag="cps")
                        for jt in range(NJT):
                            sps = spool.tile([P, QC], FP, tag="sps")
                            nc.tensor.matmul(
                                sps[:],
                                mc_(kT[pb:pb + DH, mcI, jt * P:(jt + 1) * P]),
                                mc_(qT[pb:pb + DH, mcI, qs]),
                                start=True, stop=True)
                            nc.scalar.activation(
                                fr_(e[:, jt, :]), sps[:], Exp,
                                bias=mbias[:, jt:jt + 1], scale=1.0)
                            nc.tensor.matmul(
                                cps[0:DH + 1, :], mc_(vA[:, jt, h, :]),
                                mc_(e[:, jt, :]),
                                start=(jt == 0), stop=(jt == NJT - 1))
                        rec = sm_pool.tile([P, QC], FP, tag="rec")
                        nc.vector.reciprocal(rec[0:1, :], cps[DH:DH + 1, :])
                        rb = sm_pool.tile([P, QC], FP, tag="rb")
                        nc.gpsimd.partition_broadcast(rb[:], rec[0:1, :])
                        nc.vector.tensor_mul(
                            fr_(ctxT[pb:pb + DH, mcI, qs]), cps[0:DH, :],
                            rb[0:DH, :])
                        for jt in range(NJT):
                            eo = sm_pool.tile([P, QC], FP, tag="eo")
                            nc.vector.tensor_mul(eo[:], e[:, jt, :], rb[:])
                            nc.sync.dma_start(
                                WTo[h, jt * P:(jt + 1) * P, qs], eo[:])

            # ---- Stage 3: output projection (row-sharded partial) ----
            with tc.tile_pool(name="out_sb", bufs=3) as osb, \
                 tc.tile_pool(name="opsum", bufs=4, space="PSUM") as opool:
                for sc in range(NJT):
                    for ncn in range(2):
                        ps = opool.tile([P, QC], FP, tag="ops")
                        for kc in range(2):
                            nc.tensor.matmul(
                                ps[:], mc_(ctxT[:, kc, sc * P:(sc + 1) * P]),
                                mc_(wo_sb[:, kc, ncn * QC:(ncn + 1) * QC]),
                                start=(kc == 0), stop=(kc == 1))
                        ot = osb.tile([P, QC], FP, tag="ot")
                        nc.any.tensor_copy(out=ot[:], in_=ps[:])
                        nc.sync.dma_start(
                            OUT[sc * P:(sc + 1) * P, ncn * QC:(ncn + 1) * QC],
                            ot[:])

    nc.compile()
    return nc


def _get_nc():
    if "nc" not in _CACHE:
        _CACHE["nc"] = _build_nc()
    return _CACHE["nc"]


def make_in_maps(Q, K, V, mask, Wq, bq, Wk, bk, Wv, bv, Wo, bo):
    f32 = np.float32
    QTb = [np.ascontiguousarray(np.asarray(Q, f32)[b].T) for b in range(2)]
    KTb = [np.ascontiguousarray(np.asarray(K, f32)[b].T) for b in range(2)]
    VTb = [np.ascontiguousarray(np.asarray(V, f32)[b].T) for b in range(2)]
    mask = np.asarray(mask, f32)
    Wq, Wk, Wv, Wo = (np.asarray(x, f32) for x in (Wq, Wk, Wv, Wo))
    bq, bk, bv = (np.asarray(x, f32) for x in (bq, bk, bv))
    in_maps = []
    for c in range(8):
        b, g = c // 4, c % 4
        cols = slice(g * COLS, (g + 1) * COLS)
        in_maps.append({
            "QT": QTb[b], "KT": KTb[b], "VT": VTb[b],
            "Wq": np.ascontiguousarray(Wq[:, cols]),
            "Wk": np.ascontiguousarray(Wk[:, cols]),
            "Wv": np.ascontiguousarray(Wv[:, cols]),
            "Wo": np.ascontiguousarray(Wo[cols, :]),
            "bq": np.ascontiguousarray(bq[cols]),
            "bk": np.ascontiguousarray(bk[cols]),
            "bv": np.ascontiguousarray(bv[cols]),
            "mask": np.ascontiguousarray(mask[b, 0, 0, :]),
            "ones": np.ones((P, NJT * HPC), f32),
        })
    return in_maps


def assemble(results, bo):
    f32 = np.float32
    out = np.zeros((2, S, DM), f32)
    wT = np.empty((2, H, S, S), f32)  # [b, h, key, query]
    for c, r in enumerate(results):
        b, g = c // 4, c % 4
        out[b] += r["OUT"]
        wT[b, g * HPC:(g + 1) * HPC] = r["WT"]
    out += np.asarray(bo, f32)
    weights = wT.transpose(0, 1, 3, 2)  # view: [b, h, query, key]
    return out, weights


def kernel(**inputs):
    from concourse.bass_utils import run_bass_kernel_spmd

    nc = _get_nc()
    in_maps = make_in_maps(**inputs)
    res = run_bass_kernel_spmd(nc, in_maps, core_ids=list(range(8)))
    return assemble(res.results, inputs["bo"])
